# revision 35
# baseline (speedup 1.0000x reference)
"""NT-Xent contrastive loss on 8 TRN2 NeuronCores — symmetric scheme.

Math (reference, T=0.5):
  z = l2norm(concat(query, pos))          # [8192, 256]
  sim = z @ z.T
  loss = mean_i( log(sum_{j!=i} exp(2*sim_ij)) - 2*sim_{i, i+-B} )

sim is symmetric, so each exp(2*sim_ij) is computed ONCE and credited to
both row i's and row j's denominator. 64 row-tiles of 128; core c (after
rolling x by -128*c rows) owns local row-tiles {0,8,...,56}. For owned
tile I it computes sim vs column tiles (I+d) mod 64 for d=0..32:
  d=0        diagonal tile: rowsums only (self term e^2 subtracted host-side)
  d=1..31    rowsums -> pd[:,I]; colsums -> pd[:,(I+d)%64]
  d=32       computed by both sides, rowsums only
Host sums the per-core pd surfaces (rolled back), takes ln, adds the
positives, and divides. Device never computes ln.

Per-core pipeline:
  A: DMA bf16 x -> norms on DVE -> inv=rsqrt(|x|^2/256) Newton ->
     z = x*inv as FP8 (scaled by 16) -> one DMA-engine XBAR transpose
     per 8-tile group, moving fp8 PAIRS as uint16. The resulting layout
     zT[p, t, 2j+b] = feature (2p+b) of row j in tile t is consumed by
     DoubleRow matmuls whose 2-way k-interleave absorbs the pair dim:
     lhsT/rhs APs are [128, 2, N] with strides (1, 2). No PE transposes,
     no PSUM staging, no SBUF casts.
  B: fp8 DoubleRow matmuls (K=256 in one pass) -> PSUM f32 chunks of
     1536 cols; ACT exp(scale=2/256) -> E bf16 SBUF + accum_out rowsums;
     colsums as 1-col matmuls lhsT=E_tile, rhs=ones -> PSUM strip;
     DVE adds strip into pd.
  Output [128, 72]: cols 0:64 pd surface, 64:72 s = 256*positives.
"""

import numpy as np

import concourse.bass as bass
import concourse.bacc as bacc
import concourse.tile as tile
import concourse.mybir as mybir
import concourse.bass_utils as bass_utils

F32 = mybir.dt.float32
BF16 = mybir.dt.bfloat16
FP8 = mybir.dt.float8e4
U16 = mybir.dt.uint16
AF = mybir.ActivationFunctionType
ALU = mybir.AluOpType
AXX = mybir.AxisListType.X
DR = mybir.MatmulPerfMode.DoubleRow

P = 128
D = 256
B = 4096
ROWS = 2 * B
NT = ROWS // P          # 64 row tiles
N_CORES = 8
EXP_SCALE = 2.0 / 256.0  # sim arrives x256 (z scaled by 16); T=0.5
CHUNKS = [(0, 12), (12, 12), (24, 8)]  # (tile offset, tiles) per chunk
# d=32 stragglers are batched separately (one psum tile, one exp)


def _pair(ap):
    """[P, n*256] fp8 tile slice -> DoubleRow [P, 2, n*128] (strides 1, 2)."""
    return ap.rearrange("p (j b) -> p b j", b=2)


def _phase_a(nc, t0, n, dmae, x_rt, x_sb, nsq, inv, zT, zfp, sqp, scrp,
             sqtp, n2pp, ones):
    """Normalize row-tiles [t0, t0+n) into zT (fp8, pair layout)."""
    sl = slice(t0, t0 + n)
    xg = x_sb[:, sl, :]
    dmae.dma_start(out=xg, in_=x_rt[:, sl, :])

    # |x_row|^2 via PE: square on DVE, XBAR-transpose the squares into
    # kc-layout, then per-tile colsum matmuls against ones (K=256 as two
    # accumulating 128-contractions). PE is the idle engine; this keeps
    # the big reduction off DVE.
    sq8 = sqp.tile([P, 8, D], BF16, tag="sq8")
    nc.vector.tensor_mul(sq8[:, 0:n], xg, xg)
    sqT = sqtp.tile([P, 16, P], BF16, tag="sqT")
    nc.sync.dma_start_transpose(
        sqT[:, 0:2 * n, :],
        sq8[:, 0:n].rearrange("p t d -> p (t d)"))
    n2p = n2pp.tile([P, 8], F32, tag="n2p")
    for t in range(n):
        nc.tensor.matmul(out=n2p[:, t:t + 1], lhsT=sqT[:, 2 * t, :],
                         rhs=ones, start=True, stop=False)
        nc.tensor.matmul(out=n2p[:, t:t + 1], lhsT=sqT[:, 2 * t + 1, :],
                         rhs=ones, start=False, stop=True)
    nc.vector.tensor_scalar_mul(out=nsq[:, sl], in0=n2p[:, 0:n],
                                scalar1=1.0 / 256.0)

    # inv = rsqrt(nsq) = 16/|x| via DVE Newton (linear seed + 1 iter)
    nsq_s = nsq[:, sl]
    inv_s = inv[:, sl]
    nc.vector.tensor_scalar(out=inv_s, in0=nsq_s, scalar1=-0.501,
                            scalar2=1.521, op0=ALU.mult, op1=ALU.add)
    nt_ = scrp.tile([P, 8], F32, tag="nt")
    ntn = nt_[:, 0:n]
    nc.vector.tensor_mul(ntn, inv_s, inv_s)
    nc.vector.tensor_mul(ntn, ntn, nsq_s)
    nc.vector.tensor_scalar(out=ntn, in0=ntn, scalar1=-0.5,
                            scalar2=1.5, op0=ALU.mult, op1=ALU.add)
    nc.vector.tensor_mul(inv_s, inv_s, ntn)

    # z = x * inv (16*unit rows), straight to fp8; split DVE/GPSIMD 1:3
    zf = zfp.tile([P, 8, D], FP8, tag="zf")
    h = n // 4
    nc.vector.tensor_mul(zf[:, 0:h], xg[:, 0:h],
                         inv[:, t0:t0 + h].broadcast_to([P, h, D]))
    nc.gpsimd.tensor_mul(zf[:, h:n], xg[:, h:n],
                         inv[:, t0 + h:t0 + n].broadcast_to([P, n - h, D]))

    # XBAR transpose of the whole group, fp8 pairs moved as uint16
    src = zf[:, 0:n].bitcast(U16).rearrange("p t q -> p (t q)")
    dst = zT[:, sl, :].bitcast(U16)
    nc.sync.dma_start_transpose(dst, src)


def _phase_b(nc, k, ci, zT, gramp, ep, csk, accs, ones, lhsT):
    """One 12(/9)-tile chunk of owned row-tile 8k: matmul+exp+colsums."""
    ip = 8 * k
    d0, ntiles = CHUNKS[ci]

    pt = gramp.tile([P, 1536], F32, tag="gram")
    for g in range(0, ntiles, 4):
        t0 = (ip + d0 + g) % NT
        rhs = _pair(zT[:, t0:t0 + 4, :].rearrange("p t f -> p (t f)"))
        nc.tensor.matmul(out=pt[:, g * P:(g + 4) * P], lhsT=lhsT, rhs=rhs,
                         start=True, stop=True, perf_mode=DR)

    width = ntiles * P
    e = ep.tile([P, 1536], BF16, tag="E")
    nc.scalar.activation(out=e[:, 0:width], in_=pt[:, 0:width], func=AF.Exp,
                         scale=EXP_SCALE,
                         accum_out=accs[:, 3 * k + ci:3 * k + ci + 1])

    for t in range(ntiles):
        d = d0 + t
        if 1 <= d <= 31:
            nc.tensor.matmul(out=csk[:, d - 1:d], lhsT=e[:, t * P:(t + 1) * P],
                             rhs=ones, start=True, stop=True)


def _emit(ctx, tc, nc, x_ap, y_ap):
    singles = ctx.enter_context(tc.tile_pool(name="singles", bufs=1))
    sqp = ctx.enter_context(tc.tile_pool(name="sqp", bufs=2))
    zfp = ctx.enter_context(tc.tile_pool(name="zfp", bufs=2))
    scrp = ctx.enter_context(tc.tile_pool(name="scrp", bufs=2))
    ep = ctx.enter_context(tc.tile_pool(name="ep", bufs=3))
    gramp = ctx.enter_context(tc.tile_pool(name="gramp", bufs=2, space="PSUM"))
    cstp = ctx.enter_context(tc.tile_pool(name="cstp", bufs=1, space="PSUM"))
    n2pp = ctx.enter_context(tc.tile_pool(name="n2pp", bufs=1, space="PSUM"))
    sqtp = ctx.enter_context(tc.tile_pool(name="sqtp", bufs=2))

    ones = singles.tile([P, 1], BF16)
    nc.gpsimd.memset(ones, 1.0)

    x_sb = singles.tile([P, NT, D], BF16)
    zT = singles.tile([P, NT, D], FP8)
    nsq = singles.tile([P, NT], F32)
    inv = singles.tile([P, NT], F32)
    accs = singles.tile([P, 24], F32)
    dots = singles.tile([P, 8], F32)
    out_sb = singles.tile([P, 72], F32)
    pd = out_sb[:, 0:NT]
    nc.gpsimd.memset(pd, 0.0)

    x_rt = x_ap.rearrange("(t p) d -> p t d", p=P)  # [128, 64, 256] bf16

    # A-units: 4-tile for the first 16 tiles (short first-chunk latency),
    # 8-tile after. DMA issues alternate sync/scalar to avoid one queue
    # serializing the whole input stream.
    UNITS = [(0, 4), (4, 4), (8, 4), (12, 4)] + [(16 + 8 * i, 8)
                                                 for i in range(6)]

    def a(u):
        t0, n = UNITS[u]
        dmae = nc.sync if u % 2 == 0 else nc.scalar
        _phase_a(nc, t0, n, dmae, x_rt, x_sb, nsq, inv, zT, zfp, sqp, scrp,
                 sqtp, n2pp, ones)

    cs = {}
    w2 = {}
    w2p = ctx.enter_context(tc.tile_pool(name="w2p", bufs=9))

    def b(k, ci):
        if ci == 0:
            cs_tile = cstp.tile([P, 32], F32, tag="cs")
            cs[k] = cs_tile
            # LDWEIGHTS needs a contiguous stationary: materialize the
            # own tile in pair-permutation layout [p', b, j]
            w2_tile = w2p.tile([P, 2, P], FP8, tag="w2")
            nc.vector.tensor_copy(out=w2_tile, in_=_pair(zT[:, 8 * k, :]))
            w2[k] = w2_tile
        _phase_b(nc, k, ci, zT, gramp, ep, cs[k], accs, ones, w2[k])
        if ci == 2:
            # drain colsum strip into pd with wrap split; frees cs slot
            j0 = (8 * k + 1) % NT
            n1 = min(31, NT - j0)
            nc.vector.tensor_add(out=pd[:, j0:j0 + n1], in0=pd[:, j0:j0 + n1],
                                 in1=cs[k][:, 0:n1])
            if n1 < 31:
                nc.vector.tensor_add(out=pd[:, 0:31 - n1],
                                     in0=pd[:, 0:31 - n1],
                                     in1=cs[k][:, n1:31])

    # interleave A units and B chunks by data availability so every
    # engine streams; B(k,c) needs tiles 8k+12c..8k+12c+11 (c2: +24..32)
    a(0); a(1); a(2)
    b(0, 0)
    a(3)
    a(4)
    b(0, 1); b(1, 0)
    a(5)
    b(0, 2); b(1, 1); b(2, 0)
    a(6)
    b(1, 2); b(2, 1); b(3, 0)
    a(7)
    b(2, 2); b(3, 1); b(4, 0)
    a(8)
    b(3, 2); b(4, 1); b(5, 0)
    a(9)

    # positives (own tile t vs t+32): dots, mid-stream
    xv = x_sb.rearrange("p (a b) d -> p a b d", b=8)
    sqd = sqp.tile([P, 8, D], BF16, tag="sq8")
    nc.vector.tensor_mul(sqd[:, 0:4], xv[:, 0:4, 0, :], xv[:, 4:8, 0, :])
    nc.vector.tensor_mul(sqd[:, 4:8], xv[:, 4:8, 0, :], xv[:, 0:4, 0, :])
    nc.vector.reduce_sum(out=dots, in_=sqd, axis=AXX)

    b(4, 2); b(5, 1); b(6, 0)
    b(5, 2); b(6, 1); b(7, 0)

    # d=32 stragglers, all 8 row-tiles in one psum tile + one exp.
    # Each side of the (I, I+32) pair computes it; rowsums only.
    stp = gramp.tile([P, 1536], F32, tag="gram")
    for k in range(8):
        t0 = (8 * k + 32) % NT
        nc.tensor.matmul(out=stp[:, k * P:(k + 1) * P], lhsT=w2[k],
                         rhs=_pair(zT[:, t0, :]),
                         start=True, stop=True, perf_mode=DR)
    nc.scalar.activation(out=stp[:, 0:1024], in_=stp[:, 0:1024], func=AF.Exp,
                         scale=EXP_SCALE)
    rs2 = scrp.tile([P, 8], F32, tag="rs2")
    nc.vector.reduce_sum(out=rs2,
                         in_=stp[:, 0:1024].rearrange("p (k j) -> p k j", k=8),
                         axis=AXX)

    b(6, 2); b(7, 1); b(7, 2)

    # s = dots * inv_own * inv_partner (= 256 * positive sim)
    invv = inv.rearrange("p (a b) -> p a b", b=8)
    s1 = out_sb[:, 64:72]
    nc.vector.tensor_mul(s1, dots, invv[:, :, 0])
    nc.vector.tensor_mul(s1[:, 0:4], s1[:, 0:4], invv[:, 4:8, 0])
    nc.vector.tensor_mul(s1[:, 4:8], s1[:, 4:8], invv[:, 0:4, 0])

    # own-row rowsums: accs [P, 8k, 3] + straggler rowsums -> pd[:, ::8]
    rs = scrp.tile([P, 8], F32, tag="rs")
    nc.vector.reduce_sum(out=rs, in_=accs.rearrange("p (k c) -> p k c", c=3),
                         axis=AXX)
    nc.vector.tensor_add(out=rs, in0=rs, in1=rs2)
    pdv = pd.rearrange("p (k r) -> p k r", r=8)
    nc.vector.tensor_add(out=pdv[:, :, 0], in0=pdv[:, :, 0], in1=rs)

    nc.sync.dma_start(out=y_ap, in_=out_sb)


_NC_CACHE = {}


def _get_nc():
    if "nc" not in _NC_CACHE:
        nc = bacc.Bacc("TRN2", target_bir_lowering=False, debug=False,
                       num_devices=N_CORES)
        x_ap = nc.dram_tensor("x", [ROWS, D], BF16, kind="ExternalInput").ap()
        y_ap = nc.dram_tensor("out", [P, 72], F32, kind="ExternalOutput").ap()
        from contextlib import ExitStack
        with tile.TileContext(nc) as tc, ExitStack() as ctx:
            _emit(ctx, tc, nc, x_ap, y_ap)
        nc.compile()
        _NC_CACHE["nc"] = nc
    return _NC_CACHE["nc"]


def run_device(x, trace=False, **kw):
    """x: [8192, 256] f32. Returns (list of [128,72] outs, results)."""
    import ml_dtypes
    nc = _get_nc()
    xb = np.asarray(x, dtype=np.float32).astype(ml_dtypes.bfloat16)
    in_maps = [{"x": np.ascontiguousarray(np.roll(xb, -P * c, axis=0))}
               for c in range(N_CORES)]
    res = bass_utils.run_bass_kernel_spmd(
        nc, in_maps, core_ids=list(range(N_CORES)), trace=trace, **kw)
    outs = [np.asarray(res.results[c]["out"], dtype=np.float64)
            for c in range(N_CORES)]
    return outs, res


def kernel(**inputs):
    q = np.asarray(inputs["query"], dtype=np.float32)
    p = np.asarray(inputs["pos"], dtype=np.float32)
    x = np.concatenate([q, p], axis=0)
    outs, _ = run_device(x)
    full = np.zeros((P, NT), dtype=np.float64)
    s_total = 0.0
    for c, o in enumerate(outs):
        full += np.roll(o[:, 0:NT], c, axis=1)
        s_total += o[:, NT:NT + 8].sum()
    denom = full - np.exp(2.0)
    loss = (np.log(denom).sum() - s_total / 128.0) / ROWS
    return np.float32(loss)


# revision 37
# speedup vs baseline: 1.0220x; 1.0220x over previous
"""NT-Xent contrastive loss on 8 TRN2 NeuronCores — symmetric scheme.

Math (reference, T=0.5):
  z = l2norm(concat(query, pos))          # [8192, 256]
  sim = z @ z.T
  loss = mean_i( log(sum_{j!=i} exp(2*sim_ij)) - 2*sim_{i, i+-B} )

sim is symmetric, so each exp(2*sim_ij) is computed ONCE and credited to
both row i's and row j's denominator. 64 row-tiles of 128; core c (after
rolling x by -128*c rows) owns local row-tiles {0,8,...,56}. For owned
tile I it computes sim vs column tiles (I+d) mod 64 for d=0..32:
  d=0        diagonal tile: rowsums only (self term e^2 subtracted host-side)
  d=1..31    rowsums -> pd[:,I]; colsums -> pd[:,(I+d)%64]
  d=32       computed by both sides, rowsums only
Host sums the per-core pd surfaces (rolled back), takes ln, adds the
positives, and divides. Device never computes ln.

Per-core pipeline:
  A: DMA bf16 x -> norms on DVE -> inv=rsqrt(|x|^2/256) Newton ->
     z = x*inv as FP8 (scaled by 16) -> one DMA-engine XBAR transpose
     per 8-tile group, moving fp8 PAIRS as uint16. The resulting layout
     zT[p, t, 2j+b] = feature (2p+b) of row j in tile t is consumed by
     DoubleRow matmuls whose 2-way k-interleave absorbs the pair dim:
     lhsT/rhs APs are [128, 2, N] with strides (1, 2). No PE transposes,
     no PSUM staging, no SBUF casts.
  B: fp8 DoubleRow matmuls (K=256 in one pass) -> PSUM f32 chunks of
     1536 cols; ACT exp(scale=2/256) -> E bf16 SBUF + accum_out rowsums;
     colsums as 1-col matmuls lhsT=E_tile, rhs=ones -> PSUM strip;
     DVE adds strip into pd.
  Output [128, 72]: cols 0:64 pd surface, 64:72 s = 256*positives.
"""

import numpy as np

import concourse.bass as bass
import concourse.bacc as bacc
import concourse.tile as tile
import concourse.mybir as mybir
import concourse.bass_utils as bass_utils

F32 = mybir.dt.float32
BF16 = mybir.dt.bfloat16
FP8 = mybir.dt.float8e4
U16 = mybir.dt.uint16
AF = mybir.ActivationFunctionType
ALU = mybir.AluOpType
AXX = mybir.AxisListType.X
DR = mybir.MatmulPerfMode.DoubleRow

P = 128
D = 256
B = 4096
ROWS = 2 * B
NT = ROWS // P          # 64 row tiles
N_CORES = 8
EXP_SCALE = 2.0 / 256.0  # sim arrives x256 (z scaled by 16); T=0.5
CHUNKS = [(0, 12), (12, 12), (24, 8)]  # (tile offset, tiles) per chunk
# d=32 stragglers are batched separately (one psum tile, one exp)


def _pair(ap):
    """[P, n*256] fp8 tile slice -> DoubleRow [P, 2, n*128] (strides 1, 2)."""
    return ap.rearrange("p (j b) -> p b j", b=2)


def _phase_a(nc, t0, n, x_sb, nsq, inv, zT, zfp, sqp, scrp,
             sqtp, n2p, ones):
    """Normalize row-tiles [t0, t0+n) into zT (fp8, pair layout)."""
    sl = slice(t0, t0 + n)
    xg = x_sb[:, sl, :]

    # |x_row|^2 via PE: square on DVE, XBAR-transpose the squares into
    # kc-layout, then per-tile colsum matmuls against ones (K=256 as two
    # accumulating 128-contractions). PE is the idle engine; this keeps
    # the big reduction off DVE.
    sq8 = sqp.tile([P, 8, D], BF16, tag="sq8")
    nc.vector.tensor_mul(sq8[:, 0:n], xg, xg)
    sqT = sqtp.tile([P, 16, P], BF16, tag="sqT")
    nc.sync.dma_start_transpose(
        sqT[:, 0:2 * n, :],
        sq8[:, 0:n].rearrange("p t d -> p (t d)"))
    for t in range(n):
        c = t0 + t
        nc.tensor.matmul(out=n2p[:, c:c + 1], lhsT=sqT[:, 2 * t, :],
                         rhs=ones, start=True, stop=False)
        nc.tensor.matmul(out=n2p[:, c:c + 1], lhsT=sqT[:, 2 * t + 1, :],
                         rhs=ones, start=False, stop=True)
    nc.vector.tensor_scalar_mul(out=nsq[:, sl], in0=n2p[:, sl],
                                scalar1=1.0 / 256.0)

    # inv = rsqrt(nsq) = 16/|x| via DVE Newton (linear seed + 1 iter)
    nsq_s = nsq[:, sl]
    inv_s = inv[:, sl]
    nc.vector.tensor_scalar(out=inv_s, in0=nsq_s, scalar1=-0.501,
                            scalar2=1.521, op0=ALU.mult, op1=ALU.add)
    nt_ = scrp.tile([P, 8], F32, tag="nt")
    ntn = nt_[:, 0:n]
    nc.vector.tensor_mul(ntn, inv_s, inv_s)
    nc.vector.tensor_mul(ntn, ntn, nsq_s)
    nc.vector.tensor_scalar(out=ntn, in0=ntn, scalar1=-0.5,
                            scalar2=1.5, op0=ALU.mult, op1=ALU.add)
    nc.vector.tensor_mul(inv_s, inv_s, ntn)

    # z = x * inv (16*unit rows), straight to fp8; split DVE/GPSIMD 1:3
    zf = zfp.tile([P, 8, D], FP8, tag="zf")
    h = n // 4
    nc.vector.tensor_mul(zf[:, 0:h], xg[:, 0:h],
                         inv[:, t0:t0 + h].broadcast_to([P, h, D]))
    nc.gpsimd.tensor_mul(zf[:, h:n], xg[:, h:n],
                         inv[:, t0 + h:t0 + n].broadcast_to([P, n - h, D]))

    # XBAR transpose of the whole group, fp8 pairs moved as uint16
    src = zf[:, 0:n].bitcast(U16).rearrange("p t q -> p (t q)")
    dst = zT[:, sl, :].bitcast(U16)
    nc.sync.dma_start_transpose(dst, src)


def _phase_b(nc, k, ci, zT, gramp, ep, csk, accs, ones, lhsT):
    """One 12(/9)-tile chunk of owned row-tile 8k: matmul+exp+colsums."""
    ip = 8 * k
    d0, ntiles = CHUNKS[ci]

    pt = gramp.tile([P, 1536], F32, tag="gram")
    for g in range(0, ntiles, 4):
        t0 = (ip + d0 + g) % NT
        rhs = _pair(zT[:, t0:t0 + 4, :].rearrange("p t f -> p (t f)"))
        nc.tensor.matmul(out=pt[:, g * P:(g + 4) * P], lhsT=lhsT, rhs=rhs,
                         start=True, stop=True, perf_mode=DR)

    width = ntiles * P
    e = ep.tile([P, 1536], BF16, tag="E")
    nc.scalar.activation(out=e[:, 0:width], in_=pt[:, 0:width], func=AF.Exp,
                         scale=EXP_SCALE,
                         accum_out=accs[:, 3 * k + ci:3 * k + ci + 1])

    for t in range(ntiles):
        d = d0 + t
        if 1 <= d <= 31:
            nc.tensor.matmul(out=csk[:, d - 1:d], lhsT=e[:, t * P:(t + 1) * P],
                             rhs=ones, start=True, stop=True)


def _emit(ctx, tc, nc, x_ap, y_ap):
    singles = ctx.enter_context(tc.tile_pool(name="singles", bufs=1))
    sqp = ctx.enter_context(tc.tile_pool(name="sqp", bufs=2))
    zfp = ctx.enter_context(tc.tile_pool(name="zfp", bufs=2))
    scrp = ctx.enter_context(tc.tile_pool(name="scrp", bufs=2))
    ep = ctx.enter_context(tc.tile_pool(name="ep", bufs=3))
    gramp = ctx.enter_context(tc.tile_pool(name="gramp", bufs=2, space="PSUM"))
    cstp = ctx.enter_context(tc.tile_pool(name="cstp", bufs=1, space="PSUM"))
    n2pp = ctx.enter_context(tc.tile_pool(name="n2pp", bufs=1, space="PSUM"))
    sqtp = ctx.enter_context(tc.tile_pool(name="sqtp", bufs=2))

    ones = singles.tile([P, 1], BF16)
    nc.gpsimd.memset(ones, 1.0)

    x_sb = singles.tile([P, NT, D], BF16)
    zT = singles.tile([P, NT, D], FP8)
    nsq = singles.tile([P, NT], F32)
    inv = singles.tile([P, NT], F32)
    accs = singles.tile([P, 24], F32)
    dots = singles.tile([P, 8], F32)
    out_sb = singles.tile([P, 72], F32)
    pd = out_sb[:, 0:NT]
    nc.gpsimd.memset(pd, 0.0)

    x_rt = x_ap.rearrange("(t p) d -> p t d", p=P)  # [128, 64, 256] bf16

    # Input lands in two big up-front DMAs so no per-unit copy ever
    # queues behind a dependency-gated transpose on the sync engine.
    nc.sync.dma_start(out=x_sb[:, 0:16, :], in_=x_rt[:, 0:16, :])
    nc.sync.dma_start(out=x_sb[:, 16:40, :], in_=x_rt[:, 16:40, :])
    nc.sync.dma_start(out=x_sb[:, 40:64, :], in_=x_rt[:, 40:64, :])

    # persistent psum strip for the PE-computed |x|^2 (disjoint columns)
    n2p = n2pp.tile([P, NT], F32, tag="n2p")

    # A-units: 4-tile for the first 16 tiles (short first-chunk latency),
    # 8-tile after.
    UNITS = [(0, 4), (4, 4), (8, 4), (12, 4)] + [(16 + 8 * i, 8)
                                                 for i in range(6)]

    def a(u):
        t0, n = UNITS[u]
        _phase_a(nc, t0, n, x_sb, nsq, inv, zT, zfp, sqp, scrp,
                 sqtp, n2p, ones)

    cs = {}
    w2 = {}
    w2p = ctx.enter_context(tc.tile_pool(name="w2p", bufs=9))

    def b(k, ci):
        if ci == 0:
            cs_tile = cstp.tile([P, 32], F32, tag="cs")
            cs[k] = cs_tile
            # LDWEIGHTS needs a contiguous stationary: materialize the
            # own tile in pair-permutation layout [p', b, j]
            w2_tile = w2p.tile([P, 2, P], FP8, tag="w2")
            nc.vector.tensor_copy(out=w2_tile, in_=_pair(zT[:, 8 * k, :]))
            w2[k] = w2_tile
        _phase_b(nc, k, ci, zT, gramp, ep, cs[k], accs, ones, w2[k])
        if ci == 2:
            # drain colsum strip into pd with wrap split; frees cs slot
            j0 = (8 * k + 1) % NT
            n1 = min(31, NT - j0)
            nc.vector.tensor_add(out=pd[:, j0:j0 + n1], in0=pd[:, j0:j0 + n1],
                                 in1=cs[k][:, 0:n1])
            if n1 < 31:
                nc.vector.tensor_add(out=pd[:, 0:31 - n1],
                                     in0=pd[:, 0:31 - n1],
                                     in1=cs[k][:, n1:31])

    # interleave A units and B chunks by data availability so every
    # engine streams; B(k,c) needs tiles 8k+12c..8k+12c+11 (c2: +24..32)
    a(0); a(1); a(2)
    b(0, 0)
    a(3)
    a(4)
    b(0, 1); b(1, 0)
    a(5)
    b(0, 2); b(1, 1); b(2, 0)
    a(6)
    b(1, 2); b(2, 1); b(3, 0)
    a(7)
    b(2, 2); b(3, 1); b(4, 0)
    a(8)
    b(3, 2); b(4, 1); b(5, 0)
    a(9)

    # positives (own tile t vs t+32): dots, mid-stream
    xv = x_sb.rearrange("p (a b) d -> p a b d", b=8)
    sqd = sqp.tile([P, 8, D], BF16, tag="sq8")
    nc.vector.tensor_mul(sqd[:, 0:4], xv[:, 0:4, 0, :], xv[:, 4:8, 0, :])
    nc.vector.tensor_mul(sqd[:, 4:8], xv[:, 4:8, 0, :], xv[:, 0:4, 0, :])
    nc.vector.reduce_sum(out=dots, in_=sqd, axis=AXX)

    b(4, 2); b(5, 1); b(6, 0)
    b(5, 2); b(6, 1); b(7, 0)

    # d=32 stragglers, all 8 row-tiles in one psum tile + one exp.
    # Each side of the (I, I+32) pair computes it; rowsums only.
    stp = gramp.tile([P, 1536], F32, tag="gram")
    for k in range(8):
        t0 = (8 * k + 32) % NT
        nc.tensor.matmul(out=stp[:, k * P:(k + 1) * P], lhsT=w2[k],
                         rhs=_pair(zT[:, t0, :]),
                         start=True, stop=True, perf_mode=DR)
    nc.scalar.activation(out=stp[:, 0:1024], in_=stp[:, 0:1024], func=AF.Exp,
                         scale=EXP_SCALE)
    rs2 = scrp.tile([P, 8], F32, tag="rs2")
    nc.vector.reduce_sum(out=rs2,
                         in_=stp[:, 0:1024].rearrange("p (k j) -> p k j", k=8),
                         axis=AXX)

    b(6, 2); b(7, 1); b(7, 2)

    # s = dots * inv_own * inv_partner (= 256 * positive sim)
    invv = inv.rearrange("p (a b) -> p a b", b=8)
    s1 = out_sb[:, 64:72]
    nc.vector.tensor_mul(s1, dots, invv[:, :, 0])
    nc.vector.tensor_mul(s1[:, 0:4], s1[:, 0:4], invv[:, 4:8, 0])
    nc.vector.tensor_mul(s1[:, 4:8], s1[:, 4:8], invv[:, 0:4, 0])

    # own-row rowsums: accs [P, 8k, 3] + straggler rowsums -> pd[:, ::8]
    rs = scrp.tile([P, 8], F32, tag="rs")
    nc.vector.reduce_sum(out=rs, in_=accs.rearrange("p (k c) -> p k c", c=3),
                         axis=AXX)
    nc.vector.tensor_add(out=rs, in0=rs, in1=rs2)
    pdv = pd.rearrange("p (k r) -> p k r", r=8)
    nc.vector.tensor_add(out=pdv[:, :, 0], in0=pdv[:, :, 0], in1=rs)

    nc.sync.dma_start(out=y_ap, in_=out_sb)


_NC_CACHE = {}


def _get_nc():
    if "nc" not in _NC_CACHE:
        nc = bacc.Bacc("TRN2", target_bir_lowering=False, debug=False,
                       num_devices=N_CORES)
        x_ap = nc.dram_tensor("x", [ROWS, D], BF16, kind="ExternalInput").ap()
        y_ap = nc.dram_tensor("out", [P, 72], F32, kind="ExternalOutput").ap()
        from contextlib import ExitStack
        with tile.TileContext(nc) as tc, ExitStack() as ctx:
            _emit(ctx, tc, nc, x_ap, y_ap)
        nc.compile()
        _NC_CACHE["nc"] = nc
    return _NC_CACHE["nc"]


def run_device(x, trace=False, **kw):
    """x: [8192, 256] f32. Returns (list of [128,72] outs, results)."""
    import ml_dtypes
    nc = _get_nc()
    xb = np.asarray(x, dtype=np.float32).astype(ml_dtypes.bfloat16)
    in_maps = [{"x": np.ascontiguousarray(np.roll(xb, -P * c, axis=0))}
               for c in range(N_CORES)]
    res = bass_utils.run_bass_kernel_spmd(
        nc, in_maps, core_ids=list(range(N_CORES)), trace=trace, **kw)
    outs = [np.asarray(res.results[c]["out"], dtype=np.float64)
            for c in range(N_CORES)]
    return outs, res


def kernel(**inputs):
    q = np.asarray(inputs["query"], dtype=np.float32)
    p = np.asarray(inputs["pos"], dtype=np.float32)
    x = np.concatenate([q, p], axis=0)
    outs, _ = run_device(x)
    full = np.zeros((P, NT), dtype=np.float64)
    s_total = 0.0
    for c, o in enumerate(outs):
        full += np.roll(o[:, 0:NT], c, axis=1)
        s_total += o[:, NT:NT + 8].sum()
    denom = full - np.exp(2.0)
    loss = (np.log(denom).sum() - s_total / 128.0) / ROWS
    return np.float32(loss)
